# revision 15
# baseline (speedup 1.0000x reference)
"""Grouped GEMM (MoE expert layers) on 8 Trainium2 NeuronCores.

Problem: output[s_e:e_e] = input[s_e:e_e] @ weight[e].T for 8 experts with
token counts given by expert_offsets; input [16384, 2048] f32,
weight [8, 5632, 2048] f32.

Strategy: tensor-parallel over out_features. Core c computes ALL tokens
against its contiguous 704-wide slice of OUT. The expert segmentation enters
the program only as trace-time loop bounds, which are identical on every
core, so one SPMD program serves all 8 cores. The host pre-transposes x
(-> [IN, T]) and the per-core weight slice (-> [E, IN, 704]) and converts
both to bf16: halves the dominant x-replication HBM traffic (134->67 MB per
core) and lets LDWEIGHTS use fast-weight-load. The 704-wide w stream per
(token-tile, k-tile) runs as one N=512 matmul (full PSUM bank) plus one
N=192; bf16 streams at 1 row/cycle at any N. Accumulation and output stay
fp32 (rel err ~1.6e-3 from the bf16 inputs, well under tolerance).
"""
import numpy as np
import ml_dtypes

E, IN, OUT, T, NCORES = 8, 2048, 5632, 16384, 8
OUT_C = OUT // NCORES          # 704 out-features per core
P = 128                        # partitions
KT = IN // P                   # 16 k-tiles of 128
NSPLIT = 512                   # full-PSUM-bank chunk of OUT_C
NREM = OUT_C - NSPLIT          # 192-wide remainder chunk
TT_CHUNK = 2                   # token tiles (128 tokens) per x DMA
KTH = KT // 2                  # w DMA split: kt halves for finer deps


def _pad_segments(offsets):
    """Per-expert token counts padded to multiples of P.

    Returns (sizes, padded_sizes, pad_total).
    """
    sizes = np.diff(offsets).astype(int)
    padded = [(-(-s // P)) * P for s in sizes]
    return list(sizes), padded, int(sum(padded))


def _build_program(padded_sizes, dt_in):
    import concourse.bass as bass
    import concourse.mybir as mybir
    from concourse.tile import TileContext
    from wait_legalize_embed import legalize_waits

    Tp = sum(padded_sizes)
    nc = bass.Bass()
    xT_d = nc.dram_tensor("xT", [IN, Tp], dt_in, kind="ExternalInput")
    wT_d = nc.dram_tensor("wT", [E, IN, OUT_C], dt_in, kind="ExternalInput")
    out_d = nc.dram_tensor("out", [Tp, OUT_C], mybir.dt.float32, kind="ExternalOutput")

    xT_r = xT_d.rearrange("(kt p) t -> p kt t", p=P)

    with TileContext(nc) as tc:
        with tc.tile_pool(name="wpool", bufs=4) as wpool, \
             tc.tile_pool(name="xpool", bufs=6) as xpool, \
             tc.tile_pool(name="opool", bufs=4) as opool, \
             tc.tile_pool(name="ppool", bufs=8, space="PSUM") as ppool:
            tile_base = 0
            for e in range(E):
                ntiles = padded_sizes[e] // P
                if ntiles == 0:
                    continue
                # Weights split into kt halves so the first matmul of an
                # expert waits on only half the expert slice. Expert 0 loads
                # on the scalar HWDGE queue (parallel with x0 on sync) to cut
                # startup; later experts load on sync ahead of their x chunks
                # (the x-pool WAR rotation keeps that queue several chunks
                # ahead of compute, so they arrive prefetched). Stores on the
                # scalar queue wait on PSUM evictions and would throttle any
                # w prefetch placed behind them.
                wT_e = wT_d[e].rearrange("(kt p) n -> p kt n", p=P)
                first = tile_base == 0
                if first:
                    # Eighth-granular w tiles on the scalar queue: the first
                    # matmul of the program waits on only ~0.36 MB.
                    KTQ = KT // 8
                    wq = [wpool.tile([P, KTQ, OUT_C], dt_in, tag=f"wq{q}",
                                     bufs=1, name=f"wq{q}")
                          for q in range(8)]
                    for q in range(8):
                        nc.scalar.dma_start(
                            out=wq[q][:], in_=wT_e[:, q * KTQ : (q + 1) * KTQ, :]
                        )
                    wmap = lambda kt: (wq[kt // KTQ], kt % KTQ)
                else:
                    w_lo = wpool.tile([P, KTH, OUT_C], dt_in, tag="wlo", bufs=2)
                    w_hi = wpool.tile([P, KTH, OUT_C], dt_in, tag="whi", bufs=2)
                    nc.sync.dma_start(out=w_lo[:], in_=wT_e[:, 0:KTH, :])
                    nc.sync.dma_start(out=w_hi[:], in_=wT_e[:, KTH:KT, :])
                    wmap = lambda kt: (w_lo, kt) if kt < KTH else (w_hi, kt - KTH)
                for tt0 in range(0, ntiles, TT_CHUNK):
                    cur = min(TT_CHUNK, ntiles - tt0)
                    t0 = (tile_base + tt0) * P
                    if first and tt0 == 0:
                        # kt-quartered x tiles: the first matmul waits ~0.25 MB.
                        KTX = KT // 4
                        xs = [xpool.tile([P, KTX, TT_CHUNK * P], dt_in,
                                         tag=f"xs{q}", bufs=1, name=f"xs{q}")
                              for q in range(4)]
                        for q in range(4):
                            nc.sync.dma_start(
                                out=xs[q][:, :, : cur * P],
                                in_=xT_r[:, q * KTX : (q + 1) * KTX,
                                         t0 : t0 + cur * P],
                            )
                        xmap = lambda kt: (xs[kt // KTX], kt % KTX)
                    else:
                        x_sb = xpool.tile([P, KT, TT_CHUNK * P], dt_in, tag="x")
                        nc.sync.dma_start(
                            out=x_sb[:, :, : cur * P],
                            in_=xT_r[:, :, t0 : t0 + cur * P],
                        )
                        xmap = lambda kt: (x_sb, kt)
                    o_sb = opool.tile([P, TT_CHUNK, OUT_C], mybir.dt.float32, tag="o")
                    for j in range(cur):
                        ps0 = ppool.tile([P, NSPLIT], mybir.dt.float32, tag="ps")
                        ps1 = ppool.tile([P, NREM], mybir.dt.float32, tag="ps")
                        for kt in range(KT):
                            x_t, xk = xmap(kt)
                            lhsT = x_t[:, xk, j * P : (j + 1) * P]
                            w_sb, wk = wmap(kt)
                            nc.tensor.matmul(
                                ps0[:], lhsT, w_sb[:, wk, 0:NSPLIT],
                                start=(kt == 0), stop=(kt == KT - 1),
                            )
                            nc.tensor.matmul(
                                ps1[:], lhsT, w_sb[:, wk, NSPLIT:OUT_C],
                                start=(kt == 0), stop=(kt == KT - 1),
                            )
                        nc.vector.tensor_copy(o_sb[:, j, 0:NSPLIT], ps0[:])
                        nc.vector.tensor_copy(o_sb[:, j, NSPLIT:OUT_C], ps1[:])
                    # One batched store per x chunk: out rows t0..t0+cur*128,
                    # row (j*128 + p) <- o_sb[p, j, :].
                    nc.scalar.dma_start(
                        out=out_d[t0 : t0 + cur * P, :].rearrange(
                            "(j p) n -> p j n", p=P
                        ),
                        in_=o_sb[:, :cur, :],
                    )
                tile_base += ntiles
    legalize_waits(nc)
    return nc


def _prepare(input, weight, expert_offsets):
    offs = np.asarray(expert_offsets).astype(np.int64)
    sizes, padded_sizes, Tp = _pad_segments(offs)
    x = np.asarray(input, dtype=np.float32)
    w = np.asarray(weight, dtype=np.float32)

    if Tp == T and all(s == p for s, p in zip(sizes, padded_sizes)):
        xT = np.ascontiguousarray(x.T)
    else:
        xp = np.zeros((Tp, IN), dtype=np.float32)
        base = 0
        for e in range(E):
            s, sz = int(offs[e]), sizes[e]
            xp[base : base + sz] = x[s : s + sz]
            base += padded_sizes[e]
        xT = np.ascontiguousarray(xp.T)
    xT = xT.astype(ml_dtypes.bfloat16)

    in_maps = []
    for c in range(NCORES):
        wTc = np.ascontiguousarray(
            w[:, c * OUT_C : (c + 1) * OUT_C, :].transpose(0, 2, 1)
        ).astype(ml_dtypes.bfloat16)
        in_maps.append({"xT": xT, "wT": wTc})
    return sizes, padded_sizes, Tp, in_maps


def _gather(results, sizes, padded_sizes):
    full = np.concatenate([r["out"] for r in results], axis=1)
    if sum(sizes) == full.shape[0]:
        return full
    out = np.empty((sum(sizes), OUT), dtype=np.float32)
    base_p = base = 0
    for e in range(E):
        out[base : base + sizes[e]] = full[base_p : base_p + sizes[e]]
        base += sizes[e]
        base_p += padded_sizes[e]
    return out


def run(input, weight, expert_offsets, trace=False):
    import concourse.mybir as mybir
    from concourse.bass_utils import run_bass_kernel_spmd

    sizes, padded_sizes, Tp, in_maps = _prepare(input, weight, expert_offsets)
    nc = _build_program(padded_sizes, mybir.dt.bfloat16)
    core_ids = list(range(NCORES))
    res = run_bass_kernel_spmd(nc, in_maps, core_ids, trace=trace)
    out = _gather(res.results, sizes, padded_sizes)
    return out, res


def kernel(input, weight, expert_offsets):
    out, _ = run(input, weight, expert_offsets)
    return out


# --- embedded helper (kernel.py must be self-contained) ---------------------
import sys as _sys
import types as _types

_wl_src = '''
import concourse.mybir as mybir


def legalize_waits(nc, maxw: int = 1) -> int:
    """Walrus accepts a limited number of sync-wait commands per instruction;
    split extras onto preceding same-engine NOPs (one wait each)."""
    split = 0
    for f in nc.m.functions:
        for blk in f.blocks:
            new_instructions = []
            for inst in blk.instructions:
                si = inst.sync_info
                waits = list(si.on_wait) if si and si.on_wait else []
                if len(waits) > maxw:
                    keep = waits[-maxw:]
                    extra = waits[:-maxw]
                    for w in extra:
                        nop = mybir.InstNoOp(
                            name=nc.get_next_instruction_name(),
                            sync_info=mybir.SyncInfo(on_wait=[w], on_update=[]),
                            bass_nofuse=True,
                            engine=inst.engine,
                        )
                        new_instructions.append(nop)
                        split += 1
                    inst.sync_info = mybir.SyncInfo(
                        on_wait=keep,
                        on_update=list(si.on_update) if si.on_update else [],
                    )
                new_instructions.append(inst)
            blk.instructions = new_instructions
    return split
'''

_wl_mod = _types.ModuleType("wait_legalize_embed")
exec(_wl_src, _wl_mod.__dict__)
_sys.modules["wait_legalize_embed"] = _wl_mod
